# revision 47
# baseline (speedup 1.0000x reference)
"""GAT link-prediction kernel for 8 Trainium2 NeuronCores (Bass/Tile).

Sharding: nodes split into 8 contiguous dst ranges (6250/core); real edges
(no self-loops) bucketed by (dst block of 128, src-id half) and padded so all
cores run one SPMD program. Per-layer packed node tables [rows, 384] bf16 =
[xl(256) | a_src(8) | a_dst(8) | pad]; per-edge rows fetched with dma_gather
(int16 idx, 768B rows; 1024 idx/call is the swdge ring limit). Host-built fp8
one-hot matrices turn segment softmax + scatter into PSUM matmuls. Self-loop
contributions are precomputed per layer into DRAM tables scd* =
[ee_self*xl_self | ee_self] and added per dst block with one DMA + one add —
DVE/Act op COUNT per block is the scarce resource (each op costs ~0.5us of
engine time regardless of size). exp(leaky_relu) uses the Act Lrelu+Exp
functions, with Exp writing straight into the gathered rows' a_src columns.
Layer outputs carry a factor 8 (softmax mean fold); L1 folds it into the
transposed copy's scale+bias+relu, decode folds 1/64 into the a-side copy.
Decode pairs are sharded by the a-endpoint's owner core so a-side z gathers
run from local z during the z allgather; b-side gathers follow.
"""

from contextlib import ExitStack

import numpy as np
import ml_dtypes

import concourse.bass as bass
import concourse.bacc as bacc
import concourse.mybir as mybir
import concourse.tile as tile
from concourse.bass_utils import run_bass_kernel_spmd
from concourse.masks import make_identity

P = 128
NC = 8
N = 50000
V = 5000
EL = 200000
D = 128
HID = 32
OUT = 32
H = 8
NEG = 0.2
SHARD = N // NC            # 6250
NB = (SHARD + P - 1) // P  # 49
LASTB = SHARD - (NB - 1) * P  # 106
ROW = 384
HALF = 32000
CMAX = 24                  # upper bound; actual computed per instance
ZPAD = 128                 # z row padded to 256B for gather elem minimum
VPAD = 5120
ZR = NC * NB * P           # z table rows (block-padded)
F32 = mybir.dt.float32
BF16 = mybir.dt.bfloat16
FP8 = mybir.dt.float8e4
I16 = mybir.dt.int16
I32 = mybir.dt.int32
EXP = mybir.ActivationFunctionType.Exp
RELU = mybir.ActivationFunctionType.Relu
LRELU = mybir.ActivationFunctionType.Lrelu
MULT = mybir.AluOpType.mult
ADD = mybir.AluOpType.add
MAXOP = mybir.AluOpType.max


def _wrap16(idx_flat):
    n = len(idx_flat)
    assert n % 16 == 0
    w = np.zeros((16, n // 16), np.int16)
    w[np.arange(n) % 16, np.arange(n) // 16] = idx_flat
    return np.tile(w, (8, 1))


def _plan(edge_index, x):
    src = edge_index[0].astype(np.int64)
    dst = edge_index[1].astype(np.int64)
    core = dst // SHARD
    blk = (dst % SHARD) // P
    half = (src >= HALF).astype(np.int64)

    order = np.lexsort((src, half, blk, core))
    src_s, dst_s = src[order], dst[order]

    cnt = np.zeros((NC, NB, 2), np.int64)
    np.add.at(cnt, (core[order], blk[order], half[order]), 1)
    CH = ((cnt + P - 1) // P).max(axis=0)   # [NB, 2]
    assert CH.sum(axis=1).max() <= CMAX, CH.sum(axis=1).max()
    ch_off = np.zeros((NB, 2), np.int64)
    run = 0
    for b in range(NB):
        ch_off[b, 0] = run
        run += CH[b, 0]
        ch_off[b, 1] = run
        run += CH[b, 1]
    TOTCH = int(run)
    TOTE = TOTCH * P

    flat_start = {}
    pos = 0
    for c in range(NC):
        for b in range(NB):
            for h in range(2):
                flat_start[(c, b, h)] = pos
                pos += cnt[c, b, h]

    xs = x.astype(np.int64)
    per_core = []
    for c in range(NC):
        idx1 = np.zeros(TOTE, np.int64)
        idx2 = np.zeros(TOTE, np.int64)
        dstloc = np.full(TOTE, -1, np.int64)
        for b in range(NB):
            for h in range(2):
                n_real = int(cnt[c, b, h])
                s0 = flat_start[(c, b, h)]
                seg_src = src_s[s0:s0 + n_real]
                seg_dst = dst_s[s0:s0 + n_real]
                o0 = int(ch_off[b, h]) * P
                idx1[o0:o0 + n_real] = xs[seg_src]
                idx2[o0:o0 + n_real] = seg_src - h * HALF
                dstloc[o0:o0 + n_real] = (seg_dst % SHARD) % P
        per_core.append((idx1, idx2, dstloc))
    return per_core, CH, ch_off, TOTCH, TOTE


def _build_nc(CH, ch_off, TOTCH, DG0, DG1):
    TOTE = TOTCH * P
    DCH = DG0 + DG1
    CM = int(CH.sum(axis=1).max())
    nc = bacc.Bacc("TRN2", target_bir_lowering=False, debug=False,
                   num_devices=NC, num_swdge_queues=4)

    t_embt = nc.dram_tensor("embt", [D, VPAD], BF16, kind="ExternalInput")
    t_w1 = nc.dram_tensor("w1", [D, H * HID], F32, kind="ExternalInput")
    t_w2 = nc.dram_tensor("w2", [HID, H * OUT], F32, kind="ExternalInput")
    t_a1s = nc.dram_tensor("a1s", [P, 256], F32, kind="ExternalInput")
    t_a1d = nc.dram_tensor("a1d", [P, 256], F32, kind="ExternalInput")
    t_a2s = nc.dram_tensor("a2s", [P, 256], F32, kind="ExternalInput")
    t_a2d = nc.dram_tensor("a2d", [P, 256], F32, kind="ExternalInput")
    t_b1t = nc.dram_tensor("b1t", [HID, 1], F32, kind="ExternalInput")
    t_b2x = nc.dram_tensor("b2x", [P, OUT], F32, kind="ExternalInput")
    t_idx1 = nc.dram_tensor("idx1", [P, TOTE // 16], I16, kind="ExternalInput")
    t_idx2 = nc.dram_tensor("idx2", [P, TOTE // 16], I16, kind="ExternalInput")
    t_xn = nc.dram_tensor("xn", [P, (NB * P) // 16], I16, kind="ExternalInput")
    t_ohh = nc.dram_tensor("ohh", [P, TOTE * 2], FP8, kind="ExternalInput")
    t_dia = nc.dram_tensor("dia", [P, DCH * 8], I16, kind="ExternalInput")
    t_dib = nc.dram_tensor("dib", [P, DCH * 8], I16, kind="ExternalInput")
    t_out = nc.dram_tensor("out", [P, DCH], F32, kind="ExternalOutput")

    # idx-group layout: blocks of 8 share one idx DMA
    IGB = 8
    igs = []  # (start_chunk, n_chunks) per group
    for b0 in range(0, NB, IGB):
        b1 = min(b0 + IGB, NB)
        s = int(ch_off[b0, 0])
        e = int(ch_off[b1 - 1, 1] + CH[b1 - 1, 1])
        igs.append((s, e - s))
    IGW = max(n for _, n in igs)

    rr = [0]

    def nextq():
        rr[0] = (rr[0] + 1) % 4
        return rr[0]

    with tile.TileContext(nc) as tc:
        with (
            tc.tile_pool(name="dram", bufs=1, space="DRAM") as dp,
            tc.tile_pool(name="const", bufs=1) as cp,
            tc.tile_pool(name="bpsum", bufs=2, space="PSUM") as bps,
            tc.tile_pool(name="tpsum", bufs=1, space="PSUM") as tps,
            tc.tile_pool(name="g", bufs=4) as gp,
            tc.tile_pool(name="sm", bufs=4) as sm,
            tc.tile_pool(name="psum", bufs=3, space="PSUM") as ps,
            tc.tile_pool(name="apsum", bufs=2, space="PSUM") as aps,
        ):
            est = ExitStack()
            bp = est.enter_context(tc.tile_pool(name="build", bufs=2))
            hb = est.enter_context(tc.tile_pool(name="hb", bufs=2))
            ep = est.enter_context(tc.tile_pool(name="eex", bufs=4))
            tab1 = dp.tile([VPAD, ROW], BF16)
            tab2 = dp.tile([N, ROW], BF16)
            scd1 = dp.tile([NB * P, 264], BF16)
            scd2 = dp.tile([NB * P, 264], BF16)
            h1t_d = dp.tile([HID, SHARD], BF16)
            h1t_all = dp.tile([NC * HID, SHARD], BF16)
            zloc = dp.tile([NB * P, ZPAD], BF16)
            zall_a = dp.tile([NC * 25 * P, ZPAD], BF16)
            zall_b = dp.tile([NC * (NB - 25) * P, ZPAD], BF16)

            # ---------- constants ----------
            ident = cp.tile([P, P], F32)
            make_identity(nc, ident[:])
            w1_sb = cp.tile([D, 272], F32)
            nc.sync.dma_start(out=w1_sb[:, 0:256], in_=t_w1[:, :])
            w2_sb = cp.tile([HID, 272], F32)
            nc.sync.dma_start(out=w2_sb[:, 0:256], in_=t_w2[:, :])
            b1t_sb = cp.tile([HID, 1], F32)
            nc.sync.dma_start(out=b1t_sb[:], in_=t_b1t[:, :])
            b2x_sb = cp.tile([P, OUT], F32)
            nc.sync.dma_start(out=b2x_sb[:], in_=t_b2x[:, :])

            for (t_as, t_ad, w_sb, rows) in ((t_a1s, t_a1d, w1_sb, D),
                                             (t_a2s, t_a2d, w2_sb, HID)):
                for (tt, col) in ((t_as, 256), (t_ad, 264)):
                    att = bp.tile([P, 256], F32, tag="att")
                    tmp = bp.tile([P, 256], F32, tag="atmp")
                    nc.sync.dma_start(out=att[:], in_=tt[:, :])
                    nc.vector.tensor_tensor(out=tmp[:rows], in0=w_sb[:rows, 0:256],
                                            in1=att[:rows], op=MULT)
                    nc.vector.tensor_reduce(
                        out=w_sb[:rows, col:col + 8],
                        in_=tmp[:rows].rearrange("p (h c) -> p h c", h=H),
                        axis=mybir.AxisListType.X, op=ADD)

            w1b = cp.tile([D, 272], BF16)
            nc.vector.tensor_copy(out=w1b[:], in_=w1_sb[:])
            w2b = cp.tile([HID, 272], BF16)
            nc.vector.tensor_copy(out=w2b[:], in_=w2_sb[:])

            # self-loop contribution rows [ee*xl | ee] from packed rows
            def self_chunk(srctile, cw, scd, r0):
                tes = bp.tile([P, 8, 8], F32, tag="tes")
                nc.vector.tensor_tensor(out=tes[:, 0:cw, :],
                                        in0=srctile[:, 0:cw, 256:264],
                                        in1=srctile[:, 0:cw, 264:272], op=ADD)
                te1 = bp.tile([P, 8, 8], F32, tag="te1")
                nc.scalar.activation(out=te1[:, 0:cw, :], in_=tes[:, 0:cw, :],
                                     func=EXP)
                nc.scalar.activation(out=tes[:, 0:cw, :], in_=tes[:, 0:cw, :],
                                     func=EXP, scale=NEG)
                scg = bp.tile([P, 8, 264], BF16, tag="scg")
                nc.vector.tensor_tensor(out=scg[:, 0:cw, 256:264],
                                        in0=te1[:, 0:cw, :],
                                        in1=tes[:, 0:cw, :], op=MAXOP)
                esx = bp.tile([P, 8, 256], BF16, tag="esx")
                nc.scalar.copy(
                    out=esx[:, 0:cw, :].rearrange("p c (h o) -> p c h o", h=H),
                    in_=scg[:, 0:cw, 256:264].to_broadcast([P, cw, 8, 32]))
                nc.vector.tensor_tensor(out=scg[:, 0:cw, 0:256],
                                        in0=srctile[:, 0:cw, 0:256],
                                        in1=esx[:, 0:cw, :], op=MULT)
                nc.sync.dma_start(
                    out=scd[r0:r0 + cw * P, :].rearrange("(c p) f -> p c f", p=P),
                    in_=scg[:, 0:cw, :])

            scope_build1 = nc.named_scope("build1")
            scope_build1.__enter__()
            # ---------- build table1 ----------
            embt_b = cp.tile([D, VPAD], BF16)
            nc.sync.dma_start(out=embt_b[:], in_=t_embt[:, :])
            B1G = 4
            for tv0 in range(0, VPAD // P, B1G):
                ob = bp.tile([P, B1G, 272], BF16, tag="obf")
                for j in range(B1G):
                    tv = tv0 + j
                    acc = bps.tile([P, 272], F32, space="PSUM", tag="bacc")
                    nc.tensor.matmul(out=acc[:], lhsT=embt_b[:, tv * P:(tv + 1) * P],
                                     rhs=w1b[:], start=True, stop=True)
                    nc.scalar.copy(out=ob[:, j, :], in_=acc[:])
                nc.sync.dma_start(
                    out=tab1[tv0 * P:(tv0 + B1G) * P, 0:272].rearrange(
                        "(t p) f -> p t f", p=P),
                    in_=ob[:])

            # per-local-node tab1[x[n]] rows: a_dst source + self-loop table
            xn_sb = cp.tile([P, (NB * P) // 16], I16)
            nc.sync.dma_start(out=xn_sb[:], in_=t_xn[:, :])
            an1_sb = cp.tile([P, NB, 8], BF16)
            ANW = min(8, CM)
            for c0 in range(0, NB, ANW):
                cw = min(ANW, NB - c0)
                gt = gp.tile([P, CM, ROW], BF16, tag="g")
                nc.gpsimd.dma_gather(gt[:, 0:cw, :], tab1[:, :],
                                     xn_sb[:, c0 * 8:(c0 + cw) * 8],
                                     cw * P, cw * P, ROW, queue_num=nextq())
                nc.vector.tensor_copy(out=an1_sb[:, c0:c0 + cw, :],
                                      in_=gt[:, 0:cw, 264:272])
                self_chunk(gt, cw, scd1, c0 * P)

            # ---------- edge phase ----------
            def edge_layer(tab_h0, tab_h1, t_idx, an_sb, scd, out_cb):
                tabs = (tab_h0, tab_h1)
                ig_cache = [None]

                for b in range(NB):
                    gidx = b // IGB
                    if b % IGB == 0:
                        s, n = igs[gidx]
                        idxg = bp.tile([P, IGW * 8], I16, tag="idxg")
                        nc.sync.dma_start(out=idxg[:, 0:n * 8],
                                          in_=t_idx[:, s * 8:(s + n) * 8])
                        ig_cache[0] = idxg
                    idxg = ig_cache[0]
                    gbase = igs[gidx][0]

                    scb = sm.tile([P, 264], BF16, tag="scb")
                    nc.sync.dma_start(out=scb[:], in_=scd[b * P:(b + 1) * P, :])

                    c0h = [int(ch_off[b, 0]), int(ch_off[b, 1])]
                    cws = [int(CH[b, 0]), int(CH[b, 1])]
                    C = cws[0] + cws[1]
                    base = c0h[0]
                    g = gp.tile([P, CM, ROW], BF16, tag="g")
                    for hh in (0, 1):
                        off = c0h[hh] - base
                        for s in range(0, cws[hh], 8):
                            cw = min(8, cws[hh] - s)
                            nc.gpsimd.dma_gather(
                                g[:, off + s:off + s + cw, :], tabs[hh],
                                idxg[:, (base - gbase + off + s) * 8:
                                     (base - gbase + off + s + cw) * 8],
                                cw * P, cw * P, ROW, queue_num=nextq())
                    ohb = gp.tile([P, CM, 2, P], FP8, tag="ohb")
                    nc.sync.dma_start(
                        out=ohb[:, 0:C, :, :],
                        in_=t_ohh[:, base * 2 * P:(base + C) * 2 * P].rearrange(
                            "p (c k q) -> p c k q", k=2, q=P))
                    adp = aps.tile([P, CM * 8], F32, space="PSUM", tag="adp")
                    for j in range(C):
                        nc.tensor.matmul(out=adp[:, j * 8:(j + 1) * 8],
                                         lhsT=ohb[:, j, 1, :],
                                         rhs=an_sb[:, b, :],
                                         start=True, stop=True)
                    te = sm.tile([P, CM, 8], F32, tag="te")
                    nc.vector.tensor_tensor(
                        out=te[:, 0:C, :], in0=g[:, 0:C, 256:264],
                        in1=adp[:, 0:C * 8].rearrange("p (c a) -> p c a", a=8),
                        op=ADD)
                    e1 = sm.tile([P, CM, 8], F32, tag="e1")
                    nc.scalar.activation(out=e1[:, 0:C, :], in_=te[:, 0:C, :],
                                         func=EXP)
                    nc.scalar.activation(out=te[:, 0:C, :], in_=te[:, 0:C, :],
                                         func=EXP, scale=NEG)
                    nc.vector.tensor_tensor(out=g[:, 0:C, 256:264],
                                            in0=e1[:, 0:C, :],
                                            in1=te[:, 0:C, :], op=MAXOP)
                    EH = (CM + 1) // 2
                    for (p0, p1) in ((0, min(EH, C)), (min(EH, C), C)):
                        if p1 <= p0:
                            continue
                        pc = p1 - p0
                        eex = ep.tile([P, EH, 256], BF16, tag="eex")
                        nc.scalar.copy(
                            out=eex[:, 0:pc, :].rearrange(
                                "p c (h o) -> p c h o", h=H),
                            in_=g[:, p0:p1, 256:264].to_broadcast([P, pc, 8, 32]))
                        nc.vector.tensor_tensor(
                            out=g[:, p0:p1, 0:256], in0=g[:, p0:p1, 0:256],
                            in1=eex[:, 0:pc, :], op=MULT)
                    acc = ps.tile([P, 264], F32, space="PSUM", tag="acc")
                    for j in range(C):
                        nc.tensor.matmul(out=acc[:], lhsT=ohb[:, j, 0, :],
                                         rhs=g[:, j, 0:264],
                                         start=(j == 0), stop=(j == C - 1))

                    asum = sm.tile([P, 264], F32, tag="asum")
                    nc.vector.tensor_tensor(out=asum[:], in0=acc[:], in1=scb[:],
                                            op=ADD)
                    rec = sm.tile([P, 8], F32, tag="rec")
                    nc.vector.reciprocal(out=rec[:], in_=asum[:, 256:264])
                    nc.vector.tensor_tensor(
                        out=asum[:, 0:256].rearrange("p (h o) -> p h o", h=H),
                        in0=asum[:, 0:256].rearrange("p (h o) -> p h o", h=H),
                        in1=rec[:].to_broadcast([P, 8, 32]), op=MULT)
                    hsum = sm.tile([P, OUT], F32, tag="hsum")
                    nc.vector.tensor_reduce(
                        out=hsum[:],
                        in_=asum[:, 0:256].rearrange("p (h o) -> p o h", h=H),
                        axis=mybir.AxisListType.X, op=ADD)
                    out_cb(b, hsum)

            scope_build1.__exit__(None, None, None)
            # ----- layer 1 -----
            h1t_sb = cp.tile([HID, SHARD], BF16)

            def l1_out(b, hsum):
                tp = tps.tile([HID, P], F32, space="PSUM", tag="tp")
                nc.tensor.transpose(out=tp[:], in_=hsum[:], identity=ident[:])
                w = LASTB if b == NB - 1 else P
                # h1 = relu(hsum/8 + b1), fused into the PSUM->SBUF move
                nc.scalar.activation(out=h1t_sb[:, b * P:b * P + w],
                                     in_=tp[:, 0:w], func=RELU,
                                     scale=0.125, bias=b1t_sb[:, 0:1])

            with nc.named_scope("layer1"):
                edge_layer(tab1[:, :], tab1[:, :], t_idx1, an1_sb, scd1, l1_out)
            nc.sync.dma_start(out=h1t_d[:, :], in_=h1t_sb[:])

            # a_dst2 per node from local h1t
            an2p = aps.tile([P, NB * 8], F32, space="PSUM", tag="adp")
            for t in range(NB):
                w = LASTB if t == NB - 1 else P
                nc.tensor.matmul(out=an2p[0:w, t * 8:(t + 1) * 8],
                                 lhsT=h1t_sb[:, t * P:t * P + w],
                                 rhs=w2b[:, 264:272],
                                 start=True, stop=True)
            an2_sb = cp.tile([P, NB, 8], BF16)
            nc.vector.tensor_copy(
                out=an2_sb[:, 0:NB - 1, :],
                in_=an2p[:, 0:(NB - 1) * 8].rearrange("p (c a) -> p c a", a=8))
            nc.vector.memset(an2_sb[:, NB - 1, :], 0)
            nc.vector.tensor_copy(out=an2_sb[0:LASTB, NB - 1, :],
                                  in_=an2p[0:LASTB, (NB - 1) * 8:NB * 8])

            # ----- allgather h1t -----
            nc.gpsimd.collective_compute(
                "AllGather", mybir.AluOpType.bypass,
                replica_groups=[list(range(NC))],
                ins=[h1t_d[:, :].opt()], outs=[h1t_all[:, :].opt()])

            B2H = 13  # blocks per staging piece

            # local self rows for layer 2 (tab2 rows of own shard); overlaps AG
            for s0 in range(0, NB, 8):
                sw = min(8, NB - s0)
                stage = hb.tile([P, B2H, 272], BF16, tag="stage")
                for j in range(sw):
                    tn = s0 + j
                    w = LASTB if tn == NB - 1 else P
                    acc = bps.tile([P, 272], F32, space="PSUM", tag="bacc")
                    nc.tensor.matmul(out=acc[0:w],
                                     lhsT=h1t_sb[:, tn * P:tn * P + w],
                                     rhs=w2b[:], start=True, stop=True)
                    if tn % 2 == 0:
                        nc.scalar.copy(out=stage[0:w, j, :], in_=acc[0:w])
                    else:
                        nc.vector.tensor_copy(out=stage[0:w, j, :], in_=acc[0:w])
                self_chunk(stage, sw, scd2, s0 * P)

            # ----- build table2 -----
            scope_build2 = nc.named_scope("build2")
            scope_build2.__enter__()
            for r in range(NC):
                hrt = hb.tile([HID, SHARD], BF16, tag="hrt")
                nc.sync.dma_start(out=hrt[:], in_=h1t_all[r * HID:(r + 1) * HID, :])
                for s0 in range(0, NB, B2H):
                    sw = min(B2H, NB - s0)
                    stage = hb.tile([P, B2H, 272], BF16, tag="stage")
                    for j in range(sw):
                        tn = s0 + j
                        w = LASTB if tn == NB - 1 else P
                        acc = bps.tile([P, 272], F32, space="PSUM", tag="bacc")
                        nc.tensor.matmul(out=acc[0:w],
                                         lhsT=hrt[:, tn * P:tn * P + w],
                                         rhs=w2b[:],
                                         start=True, stop=True)
                        if tn % 2 == 0:
                            nc.scalar.copy(out=stage[0:w, j, :], in_=acc[0:w])
                        else:
                            nc.vector.tensor_copy(out=stage[0:w, j, :], in_=acc[0:w])
                    full = sw - 1 if s0 + sw == NB else sw
                    if full:
                        nc.sync.dma_start(
                            out=tab2[r * SHARD + s0 * P:
                                     r * SHARD + (s0 + full) * P, 0:272].rearrange(
                                "(t p) f -> p t f", p=P),
                            in_=stage[:, 0:full, :])
                    if full != sw:
                        nc.sync.dma_start(
                            out=tab2[r * SHARD + (NB - 1) * P:(r + 1) * SHARD, 0:272],
                            in_=stage[0:LASTB, sw - 1, :])

            scope_build2.__exit__(None, None, None)
            # ----- layer 2 -----
            z_sb = cp.tile([P, NB, OUT], BF16)
            ZH = 25  # z blocks in the first allgather half

            def l2_out(b, hsum):
                # z8 = hsum + 8*b2 (8x the true z; decode scales by 1/64)
                nc.vector.tensor_tensor(out=z_sb[:, b, :], in0=hsum[:],
                                        in1=b2x_sb[:], op=ADD)
                if b == ZH - 1:
                    nc.sync.dma_start(
                        out=zloc[0:ZH * P, 0:OUT].rearrange(
                            "(c p) f -> p c f", p=P),
                        in_=z_sb[:, 0:ZH, :])
                    nc.gpsimd.collective_compute(
                        "AllGather", mybir.AluOpType.bypass,
                        replica_groups=[list(range(NC))],
                        ins=[zloc[0:ZH * P, :].opt()],
                        outs=[zall_a[:, :].opt()])

            with nc.named_scope("layer2"):
                edge_layer(tab2[0:HALF, :], tab2[HALF:N, :], t_idx2, an2_sb,
                           scd2, l2_out)
            nc.sync.dma_start(
                out=zloc[ZH * P:NB * P, 0:OUT].rearrange("(c p) f -> p c f", p=P),
                in_=z_sb[:, ZH:NB, :])

            # ----- decode: a-side gathers from local z (overlap allgather) -----
            est.close()  # release build/hb/eex pool space for the decode pools
            scope_dec = nc.named_scope("decode")
            scope_dec.__enter__()
            dst2 = ExitStack()
            dcp = dst2.enter_context(tc.tile_pool(name="dec", bufs=1))
            dgp = dst2.enter_context(tc.tile_pool(name="dgp", bufs=3))
            # second-half allgather; first half was issued mid-layer-2
            nc.gpsimd.collective_compute(
                "AllGather", mybir.AluOpType.bypass,
                replica_groups=[list(range(NC))],
                ins=[zloc[ZH * P:NB * P, :].opt()],
                outs=[zall_b[:, :].opt()])
            dia_sb = dcp.tile([P, DCH * 8], I16)
            nc.sync.dma_start(out=dia_sb[:], in_=t_dia[:, :])
            dib_sb = dcp.tile([P, DCH * 8], I16)
            nc.sync.dma_start(out=dib_sb[:], in_=t_dib[:, :])
            za32 = dcp.tile([P, DCH, OUT], BF16)
            for s in range(0, DCH, 8):
                cw = min(8, DCH - s)
                za = dgp.tile([P, 8, ZPAD], BF16, tag="zg")
                nc.gpsimd.dma_gather(za[:, 0:cw, :], zloc[:, :],
                                     dia_sb[:, s * 8:(s + cw) * 8],
                                     cw * P, cw * P, ZPAD, queue_num=nextq())
                nc.vector.tensor_scalar(out=za32[:, s:s + cw, :],
                                        in0=za[:, 0:cw, 0:OUT],
                                        scalar1=0.015625, scalar2=0.0,
                                        op0=MULT, op1=ADD)

            res = dcp.tile([P, DCH], F32)
            for gi, (g0, gch) in enumerate(((0, DG0), (DG0, DG1))):
                tz = zall_a[:, :] if gi == 0 else zall_b[:, :]
                for s in range(0, gch, 8):
                    cw = min(8, gch - s)
                    zb = dgp.tile([P, 8, ZPAD], BF16, tag="zg")
                    nc.gpsimd.dma_gather(zb[:, 0:cw, :], tz,
                                         dib_sb[:, (g0 + s) * 8:(g0 + s + cw) * 8],
                                         cw * P, cw * P, ZPAD, queue_num=nextq())
                    pr = dgp.tile([P, 8, OUT], F32, tag="pr")
                    nc.vector.tensor_tensor(out=pr[:, 0:cw, :],
                                            in0=zb[:, 0:cw, 0:OUT],
                                            in1=za32[:, g0 + s:g0 + s + cw, :],
                                            op=MULT)
                    nc.vector.tensor_reduce(out=res[:, g0 + s:g0 + s + cw],
                                            in_=pr[:, 0:cw, :],
                                            axis=mybir.AxisListType.X, op=ADD)
            nc.sync.dma_start(out=t_out[:, :], in_=res[:])
            dst2.close()
            scope_dec.__exit__(None, None, None)

    nc.compile()
    return nc


def kernel(**inputs):
    x = np.asarray(inputs["x"]).astype(np.int64)
    edge_index = np.asarray(inputs["edge_index"]).astype(np.int64)
    eli = np.asarray(inputs["edge_label_index"]).astype(np.int64)
    emb = np.asarray(inputs["emb"]).astype(np.float32)
    W1 = np.asarray(inputs["W1"]).astype(np.float32)
    W2 = np.asarray(inputs["W2"]).astype(np.float32)
    a1s = np.asarray(inputs["att_src1"]).astype(np.float32).reshape(-1)
    a1d = np.asarray(inputs["att_dst1"]).astype(np.float32).reshape(-1)
    a2s = np.asarray(inputs["att_src2"]).astype(np.float32).reshape(-1)
    a2d = np.asarray(inputs["att_dst2"]).astype(np.float32).reshape(-1)
    b1 = np.asarray(inputs["b1"]).astype(np.float32).reshape(-1)
    b2 = np.asarray(inputs["b2"]).astype(np.float32).reshape(-1)

    per_core, CH, ch_off, TOTCH, TOTE = _plan(edge_index, x)

    # ---- decode plan: pairs sharded by owner(a); groups by half(zrow(b)) ----
    a_all, b_all = eli[0], eli[1]
    owner = a_all // SHARD
    ZH = 25
    b_r = b_all // SHARD
    b_rem = b_all % SHARD
    grp = (b_rem >= ZH * P).astype(np.int64)
    zrow = np.where(grp == 0, b_r * (ZH * P) + b_rem,
                    b_r * ((NB - ZH) * P) + b_rem - ZH * P)
    core_plans = []
    g_counts = np.zeros((NC, 2), np.int64)
    for c in range(NC):
        sel = np.nonzero(owner == c)[0]
        order = sel[np.argsort(grp[sel], kind="stable")]
        n0 = int((grp[sel] == 0).sum())
        core_plans.append((order, n0))
        g_counts[c, 0] = n0
        g_counts[c, 1] = len(sel) - n0
    DG0 = int((g_counts[:, 0].max() + P - 1) // P)
    DG1 = int((g_counts[:, 1].max() + P - 1) // P)
    DCH = DG0 + DG1

    emb_pad = np.zeros((VPAD, D), np.float32)
    emb_pad[:V] = emb
    common = {
        "embt": np.ascontiguousarray(emb_pad.T).astype(ml_dtypes.bfloat16),
        "w1": W1, "w2": W2,
        "a1s": np.tile(a1s, (P, 1)), "a1d": np.tile(a1d, (P, 1)),
        "a2s": np.tile(a2s, (P, 1)), "a2d": np.tile(a2d, (P, 1)),
        "b1t": b1.reshape(HID, 1), "b2x": np.tile(8.0 * b2, (P, 1)),
    }
    in_maps = []
    out_perms = []
    ei = np.arange(TOTE)
    for c in range(NC):
        idx1, idx2, dstloc = per_core[c]
        ohh = np.zeros((P, TOTCH, 2, P), np.uint8)
        real = dstloc >= 0
        ohh[ei[real] % P, ei[real] // P, 0, dstloc[real]] = 0x38
        ohh[:, :, 1, :] = ohh[:, :, 0, :].transpose(2, 1, 0)
        xn_ids = np.zeros(NB * P, np.int64)
        xn_ids[:SHARD] = x[c * SHARD:(c + 1) * SHARD]

        order, n0 = core_plans[c]
        n1 = len(order) - n0
        dia = np.zeros(DCH * P, np.int64)
        dib = np.zeros(DCH * P, np.int64)
        pm = np.full(DCH * P, -1, np.int64)
        dia[:n0] = a_all[order[:n0]] % SHARD
        dib[:n0] = zrow[order[:n0]]
        pm[:n0] = order[:n0]
        o1 = DG0 * P
        dia[o1:o1 + n1] = a_all[order[n0:]] % SHARD
        dib[o1:o1 + n1] = zrow[order[n0:]]
        pm[o1:o1 + n1] = order[n0:]
        out_perms.append(pm)

        m = dict(common)
        m["idx1"] = _wrap16(idx1.astype(np.int16))
        m["idx2"] = _wrap16(idx2.astype(np.int16))
        m["xn"] = _wrap16(xn_ids.astype(np.int16))
        m["ohh"] = ohh.reshape(P, TOTE * 2).view(ml_dtypes.float8_e4m3)
        m["dia"] = _wrap16(dia.astype(np.int16))
        m["dib"] = _wrap16(dib.astype(np.int16))
        in_maps.append(m)

    nc = _build_nc(CH, ch_off, TOTCH, DG0, DG1)
    import os
    trace = bool(int(os.environ.get("GAT_TRACE", "0")))
    if trace:
        try:
            import sys as _sys, types as _types
            import antenv as _antenv
            from trn_agent_boot.trn_boot import _ntff_profile_via_ctypes as _np_hook
            _hm = _types.ModuleType("antenv.axon_hooks")
            _hm.get_axon_ntff_profile_hook = (
                lambda: _np_hook('/opt/axon/libaxon_pjrt.so'))
            _hm.set_axon_ntff_profile_hook = lambda h: None
            _sys.modules["antenv.axon_hooks"] = _hm
            _antenv.axon_hooks = _hm
        except Exception:
            trace = False
    r = run_bass_kernel_spmd(nc, in_maps, core_ids=list(range(NC)), trace=trace)
    if trace and r.exec_time_ns:
        print("HW exec time: %d ns" % r.exec_time_ns)
        if r.per_core_scope_times:
            for s, m in sorted(r.per_core_scope_times.items()):
                print("  scope %-8s %s" % (s, {k: "%dus" % (v // 1000) for k, v in m.items()}))
        if r.instructions_and_trace:
            print("trace:", r.instructions_and_trace[1])

    out = np.zeros(EL, np.float32)
    for c in range(NC):
        res = r.results[c]["out"]
        pm = out_perms[c]
        vals = res.T.reshape(-1)
        valid = pm >= 0
        out[pm[valid]] = vals[valid]
    return out


if __name__ == "__main__":
    d = np.load("/root/problem/ref_data.npz")
    inputs = {k: d[k] for k in ("x", "edge_index", "edge_label_index", "emb",
                                "W1", "att_src1", "att_dst1", "b1",
                                "W2", "att_src2", "att_dst2", "b2")}
    got = kernel(**inputs)
    exp = d["expected"]
    denom = np.abs(exp).mean()
    rel = np.abs(got - exp) / denom
    print("Relative error: max %.3e mean %.3e" % (rel.max(), rel.mean()))
